# revision 16
# baseline (speedup 1.0000x reference)
import sys

sys.path.insert(0, "/opt/trn_rl_repo")

import numpy as np

from concourse import bass, mybir, tile
from concourse import bass_utils

B, N, K, D = 4, 16384, 32, 64
HALF = 8192             # points per core
PB = HALF // 2          # 4096 points per block (A = partitions 0-63, B = 64-127)
M2 = PB * K             # 131072 pair-columns per core
UNIT = 1024             # columns per pipeline unit (2 PSUM banks of f32)
CHUNK = 4096            # columns per DMA chunk
NU = CHUNK // UNIT      # 4 units per chunk
NCHUNK = M2 // CHUNK    # 32

TRACE = False
LAST_RESULTS = None

_BUILT = None
_MULT_SCAN = None


def _get_mult_scan_op():
    """Fused (in0 * in1) running prefix-sum along the free dim, one DVE pass.
    K-segment sums are recovered as differences of the prefix at stride K."""
    global _MULT_SCAN
    if _MULT_SCAN is not None:
        return _MULT_SCAN
    import concourse.dve_ops as dve_ops
    from concourse.dve_spec import Spec, Src0, Src1, AluOp, scan, lower
    from concourse.dve_uop import DveOpSpec

    name = "TT_MULT_PREFIX_ANT"
    for op in dve_ops.OPS:
        if op.name == name:
            _MULT_SCAN = op
            return op
    spec = Spec(
        body=scan(AluOp.ADD, Src0 * Src1),
        reference=lambda in0, in1: np.cumsum(
            in0.astype(np.float32) * in1.astype(np.float32), axis=-1
        ),
    )
    shas = {ver: DveOpSpec(name=name, opcode=0, uops=lower(spec, ver=ver),
                           rd1_en=True).sha(ver)
            for ver in ("v3", "v4")}
    op = dve_ops.DveOp(name, spec, subdim=False, uops_sha=shas)
    dve_ops.OPS.append(op)
    dve_ops._SUB_OPCODE_FOR_NAME[name] = (
        dve_ops._CUSTOM_DVE_ROW_BASE + len(dve_ops.OPS) - 1
    )
    dve_ops.CUSTOM_DVE_SPECS[name] = spec
    _MULT_SCAN = op
    return op


def _build():
    f32 = mybir.dt.float32
    f16 = mybir.dt.float16
    Prelu = mybir.ActivationFunctionType.Prelu
    sub = mybir.AluOpType.subtract
    ms_op = _get_mult_scan_op()

    nc = bass.Bass()
    xgT_d = nc.declare_dram_parameter("xgT", [128, M2 // UNIT, UNIT], f16, False)
    # rel rows live on 4 partition bands (0-7, 32-39, 64-71, 96-103) so the
    # W1 matmul runs as 4 concurrent 32-row PE tiles (~3-4x column rate).
    relb_d = nc.declare_dram_parameter("relb", [32, NCHUNK, 2, 512], f16, False)
    W1_d = nc.declare_dram_parameter("W1blk", [128, 128], f16, False)
    b1_d = nc.declare_dram_parameter("b1blk", [128, 1], f32, False)
    W2_d = nc.declare_dram_parameter("W2blk", [128, 128], f16, False)
    out_d = nc.declare_dram_parameter("out", [128, NCHUNK, NU, UNIT // K],
                                      f32, True)

    with tile.TileContext(nc) as tc:
        frees = []

        def T(shape, dtype, name):
            t, f = tc.tile(shape, dtype, name=name)
            frees.append(f)
            return t

        W1_sb = T([128, 128], f16, "W1_sb")
        b1_sb = T([128, 1], f32, "b1_sb")
        W2_sb = T([128, 128], f16, "W2_sb")
        acc_sb = T([128, NCHUNK, NU, UNIT // K], f32, "acc_sb")
        act_pre = T([128, 1], f16, "act_pre")
        nc.sync.dma_start(W1_sb[:, :], W1_d[:, :])
        nc.sync.dma_start(b1_sb[:, :], b1_d[:, :])
        nc.sync.dma_start(W2_sb[:, :], W2_d[:, :])
        # Preload the activation table before the pipeline starts so the
        # first real activation doesn't stall ~1.3us mid-stream.
        nc.scalar.activation(act_pre[:, :], W2_sb[:, 0:1], Prelu,
                             bias=b1_sb[:, :], alpha=0.1)

        with tc.tile_pool(name="xpool", bufs=3) as xpl, \
             tc.tile_pool(name="rpool", bufs=3) as rpl, \
             tc.tile_pool(name="upool", bufs=2, space="PSUM") as upl, \
             tc.tile_pool(name="wpool", bufs=2, space="PSUM") as wpl, \
             tc.tile_pool(name="vpool", bufs=8) as vpl, \
             tc.tile_pool(name="ppool", bufs=2) as ppl:
            # Short HAM warm-up overlapping the chunk-0 DMA latency.
            warm = upl.tile([128, UNIT], f32, name="u")
            for j in range(6):
                nc.tensor.matmul(warm[:, 0:128], lhsT=W2_sb[:, :],
                                 rhs=W2_sb[:, :], start=True, stop=True)

            NG = NCHUNK * (NU // 2)      # 64 g-groups of 2 units
            chunk_state = {}
            pending = None               # (vs, ws_slots..., pf, g, c) skew-1

            def issue_w2_and_scan(st):
                vs, pf, g, c, xg_t = st
                ws = []
                for i in range(2):
                    w = wpl.tile([128, UNIT], f32, name="w")
                    nc.tensor.matmul(w[:, 0:512], lhsT=W2_sb[:, :],
                                     rhs=vs[i][:, 0:512],
                                     start=True, stop=True)
                    nc.tensor.matmul(w[:, 512:1024], lhsT=W2_sb[:, :],
                                     rhs=vs[i][:, 512:1024],
                                     start=True, stop=True)
                    ws.append(w)
                for i, h in enumerate((2 * g, 2 * g + 1)):
                    nc.vector._custom_dve(ms_op,
                                          out=pf[:, h, 1:UNIT + 1],
                                          in0=ws[i][:, :],
                                          in1=xg_t[:, h, :])
                if g == NU // 2 - 1:
                    nc.gpsimd.tensor_tensor(acc_sb[:, c, :, :],
                                            pf[:, :, K:UNIT + 1:K],
                                            pf[:, :, 0:UNIT + 1 - K:K], sub)
                    if c % 2 == 1:
                        nc.sync.dma_start(out_d[:, c - 1:c + 1, :, :],
                                          acc_sb[:, c - 1:c + 1, :, :])

            for G in range(NG):
                c, g = divmod(G, NU // 2)
                if g == 0:
                    xg_t = xpl.tile([128, NU, UNIT], f16, name="xg")
                    rl_t = rpl.tile([128, 2, 512], f16, name="rl")
                    # rel first: the W1 wave (pipeline head) needs it before
                    # the bulk xg data (consumed ~3us later by the scans).
                    for bnd in range(4):
                        nc.sync.dma_start(rl_t[32 * bnd:32 * bnd + 8, :, :],
                                          relb_d[8 * bnd:8 * bnd + 8, c, :, :])
                    if c == 0:
                        for uu in range(NU):
                            nc.sync.dma_start(xg_t[:, uu, :],
                                              xgT_d[:, uu, :])
                    else:
                        nc.sync.dma_start(xg_t[:, :, :],
                                          xgT_d[:, c * NU:(c + 1) * NU, :])
                    pf = ppl.tile([128, NU, UNIT + 1], f32, name="pf")
                    nc.gpsimd.memset(pf[:, :, 0:1], 0.0)
                    chunk_state[c] = (xg_t, rl_t, pf)
                xg_t, rl_t, pf = chunk_state[c]
                # One wave of 4 concurrent row-tiled W1 matmuls fills both
                # u tiles (4 distinct PSUM banks) in ~1/4 the column time.
                # Issued one group AHEAD of the W2 matmuls (skew-1 software
                # pipeline) so the PE never idles waiting on Act latency.
                u0 = upl.tile([128, UNIT], f32, name="u")
                u1 = upl.tile([128, UNIT], f32, name="u")
                for bnd, (ut, sl) in enumerate(
                        ((u0, slice(0, 512)), (u0, slice(512, 1024)),
                         (u1, slice(0, 512)), (u1, slice(512, 1024)))):
                    p = 32 * bnd
                    nc.tensor.matmul(ut[:, sl],
                                     lhsT=W1_sb[p:p + 8, :],
                                     rhs=rl_t[p:p + 8, g, :],
                                     start=True, stop=True,
                                     tile_position=(p, 0))
                vs = []
                for i in range(2):
                    v = vpl.tile([128, UNIT], f16, name="v")
                    nc.scalar.activation(v[:, :], [u0, u1][i][:, :], Prelu,
                                         bias=b1_sb[:, :], alpha=0.1)
                    vs.append(v)
                if pending is not None:
                    issue_w2_and_scan(pending)
                pending = (vs, pf, g, c, xg_t)
            issue_w2_and_scan(pending)
        for f in reversed(frees):
            f()

    import bass_rust
    bass_rust.move_matmul_waits_to_ldweights(nc.m)
    bass_rust.generate_event_semaphores(nc)
    mybir.codegen_inst_isa_subclasses(nc)
    return nc


def _get_nc():
    global _BUILT
    if _BUILT is None:
        _BUILT = _build()
    return _BUILT


def _prep_core(x, pos, nidx, c, W1blk, W2blk, b1blk):
    b, hh = c // 2, c % 2
    sl = slice(hh * HALF, (hh + 1) * HALF)
    idxh = nidx[b, sl]                                 # [HALF, K]
    xg = x[b][idxh]                                    # [HALF, K, 64]
    rel = pos[b, sl][:, None, :] - pos[b][idxh]        # [HALF, K, 3]
    xgT = np.empty((128, M2), np.float16)
    xgT[0:64] = xg[:PB].reshape(M2, 64).T
    xgT[64:128] = xg[PB:].reshape(M2, 64).T
    rel8 = np.zeros((8, M2), np.float16)
    rel8[0:3] = rel[:PB].reshape(M2, 3).T
    rel8[4:7] = rel[PB:].reshape(M2, 3).T
    # band-interleave: band bnd holds chunk-cols g*2048 + bnd*512 + j on
    # SBUF partitions 32*bnd .. 32*bnd+7
    relb = rel8.reshape(8, NCHUNK, 2, 4, 512).transpose(3, 0, 1, 2, 4)
    return dict(xgT=np.ascontiguousarray(xgT.reshape(128, M2 // UNIT, UNIT)),
                relb=np.ascontiguousarray(relb.reshape(32, NCHUNK, 2, 512)),
                W1blk=W1blk, W2blk=W2blk, b1blk=b1blk)


def kernel(x, pos, neighbor_idx, W1, b1, W2, b2):
    nc = _get_nc()
    W1blk8 = np.zeros((8, 128), np.float16)
    W1blk8[0:3, 0:64] = W1
    W1blk8[4:7, 64:128] = W1
    W1blk = np.zeros((128, 128), np.float16)
    for bnd in range(4):
        W1blk[32 * bnd:32 * bnd + 8] = W1blk8
    b1blk = np.concatenate([b1, b1]).astype(np.float32).reshape(128, 1)
    W2blk = np.zeros((128, 128), np.float16)
    W2blk[0:64, 0:64] = W2
    W2blk[64:128, 64:128] = W2
    in_maps = [_prep_core(x, pos, neighbor_idx, c, W1blk, W2blk, b1blk)
               for c in range(8)]
    global LAST_RESULTS
    res = bass_utils.run_bass_kernel_spmd(nc, in_maps, list(range(8)),
                                          trace=TRACE)
    LAST_RESULTS = res
    out = np.empty((B, N, D), np.float32)
    for c in range(8):
        b, hh = c // 2, c % 2
        r = np.asarray(res.results[c]["out"]).reshape(128, PB)
        out[b, hh * HALF:hh * HALF + PB] = r[0:64].T
        out[b, hh * HALF + PB:(hh + 1) * HALF] = r[64:128].T
    if np.any(b2):
        for b in range(B):
            s = x[b][neighbor_idx[b]].sum(axis=1)
            out[b] += b2[None, :] * s
    return out
